# revision 44
# baseline (speedup 1.0000x reference)
"""MultiHeadDiffAttention Trainium2 kernel (8 NeuronCores), v3.

Sharding: batch (4) x head-group (2 groups of 8 heads) = 8 cores.
Each core computes a partial (T, C) c_proj output for its batch element
restricted to its 8 heads; the host sums the two head-group partials
(fp16 partials, fp32 host sum).

~230us HW (v1 baseline: ~264us). All matmuls fp16 (fp8 DoubleRow was
tried for Q/K projections: 2x PE rate but the differential-attention
cancellation amplifies quantization noise ~4x -> 3.7e-2 rel err, FAIL).

Key structure:
  - Scores per (stream, k-tile j) into one [P,2,1024] f32 PSUM tile
    (stream s in banks 2s..2s+1), ONE merged exp ACT per j (strided
    2-stream AP for j<4) -> 8-15 exps/head instead of 24; P chunks
    stored tightly per-j ([P,2,1024-128j] fp16, q-offset layout for
    j>=4).
  - Dual-stream trick: PV with a ones column yields Y_s and softmax
    denominator den_s per stream; z = Y1 - (lam*den1/den2)*Y2 equals
    den1*(a1 - lam*a2)@V, and LN scale-invariance (eps scaled by
    den1^2, folded into the Ln/Exp chain with bias ln(1-lambda))
    reproduces the reference exactly.
  - PV results staged PSUM->SBUF fp16 immediately (frees banks, the
    combine is batched per head on DVE with a single recip/gneg batch).
  - LN stats (bn_stats/aggr) + veps + apply batched per head, run in
    the main loop two heads behind; per-head PE transposes (fp16
    identity) three heads behind -> tiny tail.
  - Tail: PV(7) with per-tile combine+LN finish, head-7 transposes per
    half interleaved with c_proj (c_proj PSUM alternates psS/psB2
    banks); fp16 output DMA per q-tile on alternating queues.
  - In-order PE queue discipline: warm-up dummies sized to the pstate
    ramp (~3.5us) only; input DMAs ordered wq/wk head0 -> xT -> wv
    split across sync/scalar/gpsimd queues.
"""

import contextlib
import ctypes
import math
import sys
import types

import numpy as np

sys.path.insert(0, "/opt/trn_rl_repo")


def _install_ntff_hook():
    """Provide antenv.axon_hooks if the image lacks it (for trace=True)."""
    try:
        from antenv.axon_hooks import get_axon_ntff_profile_hook  # noqa: F401

        return
    except ImportError:
        pass

    so_path = "/opt/axon/libaxon_pjrt.so"

    def _make_hook():
        try:
            lib = ctypes.CDLL(so_path)
        except OSError:
            return None
        if not hasattr(lib, "axon_start_nrt_profile"):
            return None
        lib.axon_start_nrt_profile.argtypes = [
            ctypes.POINTER(ctypes.c_int64),
            ctypes.c_size_t,
        ]
        lib.axon_start_nrt_profile.restype = ctypes.c_int64
        lib.axon_stop_nrt_profile.argtypes = [ctypes.c_char_p]
        lib.axon_stop_nrt_profile.restype = ctypes.c_int64

        @contextlib.contextmanager
        def _hook(output_dir, device_ids):
            import jax

            jax.devices()
            if device_ids:
                ids = (ctypes.c_int64 * len(device_ids))(*device_ids)
                rc = lib.axon_start_nrt_profile(ids, len(device_ids))
            else:
                rc = lib.axon_start_nrt_profile(None, 0)
            if rc != 0:
                raise RuntimeError(f"axon_start_nrt_profile rc={rc}")
            try:
                yield
            finally:
                n = lib.axon_stop_nrt_profile(str(output_dir).encode())
                if n < 0:
                    raise RuntimeError(f"axon_stop_nrt_profile rc={n}")

        return _hook

    mod = types.ModuleType("antenv.axon_hooks")
    _the_hook = _make_hook()
    mod.get_axon_ntff_profile_hook = lambda: _the_hook
    sys.modules["antenv.axon_hooks"] = mod


_install_ntff_hook()

import ml_dtypes  # noqa: E402

import concourse.bass as bass  # noqa: E402
import concourse.mybir as mybir  # noqa: E402
import concourse.tile as tile  # noqa: E402
from concourse.masks import make_identity  # noqa: E402

P = 128
T = 1024
C = 1024
NH = 8  # heads per core
HS = 64
LAMBDA_INIT = 0.8 - 0.6 * math.exp(-0.3 * (2 - 1))
LN_EPS = 1e-5
N_CORES = 8
SCORE_SCALE = 0.125

f32 = mybir.dt.float32
f16 = mybir.dt.float16
f8 = mybir.dt.float8e4
Alu = mybir.AluOpType
Act = mybir.ActivationFunctionType
DR = mybir.MatmulPerfMode.DoubleRow


def build_program():
    nc = bass.Bass()
    xt_d = nc.dram_tensor("xt", [C, T], f16, kind="ExternalInput")
    wq_d = nc.dram_tensor("wq", [C, C], f16, kind="ExternalInput")
    wk_d = nc.dram_tensor("wk", [C, C], f16, kind="ExternalInput")
    wv_d = nc.dram_tensor("wv", [C, C], f16, kind="ExternalInput")
    wc_d = nc.dram_tensor("wc", [C, C], f16, kind="ExternalInput")
    lamneg_d = nc.dram_tensor("lamneg", [P, NH], f32, kind="ExternalInput")
    out_d = nc.dram_tensor("out", [T, C], f16, kind="ExternalOutput")

    ln_bias = float(math.log(1.0 - LAMBDA_INIT))

    with tile.TileContext(nc) as tc:
        with (
            tc.tile_pool(name="const", bufs=1) as const,
            tc.tile_pool(name="ydata", bufs=1) as y_pool,
            tc.tile_pool(name="vdata", bufs=8) as v_p,
            tc.tile_pool(name="ylnTp", bufs=1) as ylnT_p,
        ):
            ident16 = const.tile([P, P], f16, tag="ident")
            make_identity(nc, ident16)
            lamneg = const.tile([P, NH], f32, tag="lamneg")
            den_s = const.tile([P, NH, 8], f32, tag="den")
            muvar = const.tile([P, NH, 8, 2], f32, tag="muvar")
            inv_s = const.tile([P, NH, 8], f32, tag="inv")
            lnb = const.tile([P, 1], f32, tag="lnb")
            nc.vector.memset(lnb, ln_bias)
            dum = const.tile([P, 512], f16, tag="dum")
            nc.vector.memset(dum, 0.0)

            y_all = y_pool.tile([P, 8, NH, P], f16, tag="y", name="yall")
            v_aug = [
                v_p.tile([P, NH, 130], f16, tag="v", name="vaug") for _ in range(8)
            ]
            ylnT = ylnT_p.tile([P, NH, T], f16, tag="ylnT")

            yst_ctx = tc.tile_pool(name="ystp", bufs=1)
            yst_p = yst_ctx.__enter__()
            p_ctx = tc.tile_pool(name="pprob", bufs=2)
            p_pool = p_ctx.__enter__()
            small_ctx = tc.tile_pool(name="smallc", bufs=4)
            small = small_ctx.__enter__()

            def pv_unit(h, s, i, pcs, ypool, yst, ypair):
                """PV for one (stream, q-tile): accumulate over k-tiles with
                the ones column giving the softmax denominator, then stage
                the PSUM result to SBUF fp16. Both streams of a q-tile
                share one [P,2,129] PSUM tile so 2 pool bufs give pipeline
                depth 4."""
                yp = ypool.tile([P, 129], f32, tag="psY", name="yp")
                for j in range(i + 1):
                    pj = pcs[j]
                    lo = 128 * (i - j) if j < 4 else 128 * i - 512
                    nc.tensor.matmul(
                        yp,
                        lhsT=pj[:, s, lo : lo + 128],
                        rhs=v_aug[j][:, h, 0:129],
                        start=(j == 0),
                        stop=(j == i),
                    )
                nc.vector.tensor_copy(out=yst[:, i, s, 0:129], in_=yp)

            def cmb_smalls(g, yst):
                """Batched per-head: gneg_i = -lam*den1_i/den2_i, and den1
                saved for the veps chain."""
                nc.vector.tensor_copy(
                    out=den_s[:, g, :], in_=yst[:, :, 0, 128:129]
                )
                r2 = small.tile([P, 8], f32, tag="r2", name="r2")
                nc.vector.reciprocal(out=r2, in_=yst[:, :, 1, 128:129])
                nc.vector.tensor_mul(out=r2, in0=r2, in1=den_s[:, g, :])
                gneg = small.tile([P, 8], f32, tag="gneg", name="gneg")
                nc.vector.tensor_scalar(
                    out=gneg, in0=r2, scalar1=lamneg[:, g : g + 1],
                    scalar2=None, op0=Alu.mult,
                )
                return gneg

            def cmb_unit(g, i, yst, gneg):
                """z_i = y1_i + gneg_i * y2_i -> y_all."""
                tmp = small.tile([P, P], f16, tag="tmp", name="tmp")
                nc.vector.tensor_scalar(
                    out=tmp, in0=yst[:, i, 1, 0:128],
                    scalar1=gneg[:, i : i + 1], scalar2=None, op0=Alu.mult,
                )
                nc.vector.tensor_add(
                    out=y_all[:, i, g, :], in0=yst[:, i, 0, 0:128], in1=tmp
                )

            def ln_fin(g, i):
                """Per-tile veps -> inv -> LN apply (tail fast path)."""
                veps = small.tile([P, 1], f32, tag="veps1", name="veps1")
                nc.vector.tensor_mul(
                    out=veps, in0=den_s[:, g, i : i + 1],
                    in1=den_s[:, g, i : i + 1],
                )
                nc.vector.tensor_scalar(
                    out=veps, in0=veps, scalar1=LN_EPS, scalar2=None,
                    op0=Alu.mult,
                )
                nc.vector.tensor_add(
                    out=veps, in0=veps, in1=muvar[:, g, i, 1:2]
                )
                ig = inv_s[:, g, i : i + 1]
                nc.scalar.activation(out=ig, in_=veps, func=Act.Ln)
                nc.scalar.activation(
                    out=ig, in_=ig, func=Act.Exp, scale=-0.5, bias=lnb
                )
                nc.vector.tensor_scalar(
                    out=y_all[:, i, g, :],
                    in0=y_all[:, i, g, :],
                    scalar1=muvar[:, g, i, 0:1],
                    scalar2=ig,
                    op0=Alu.subtract,
                    op1=Alu.mult,
                )

            def cmb_now(g, i, yst):
                """Per-tile combine + LN stats (tail path: no batching so
                the chain overlaps the remaining PV units)."""
                nc.vector.tensor_copy(
                    out=den_s[:, g, i : i + 1], in_=yst[:, i, 0, 128:129]
                )
                r2 = small.tile([P, 1], f32, tag="r2", name="r2")
                nc.vector.reciprocal(out=r2, in_=yst[:, i, 1, 128:129])
                gneg = small.tile([P, 1], f32, tag="gneg", name="gneg")
                nc.vector.tensor_scalar(
                    out=gneg, in0=r2, scalar1=den_s[:, g, i : i + 1],
                    scalar2=lamneg[:, g : g + 1], op0=Alu.mult, op1=Alu.mult,
                )
                tmp = small.tile([P, P], f16, tag="tmp", name="tmp")
                nc.vector.tensor_scalar(
                    out=tmp, in0=yst[:, i, 1, 0:128],
                    scalar1=gneg, scalar2=None, op0=Alu.mult,
                )
                nc.vector.tensor_add(
                    out=y_all[:, i, g, :], in0=yst[:, i, 0, 0:128], in1=tmp
                )
                bs = small.tile(
                    [P, nc.vector.BN_STATS_DIM], f32, tag="bs1", name="bs1"
                )
                nc.vector.bn_stats(out=bs, in_=y_all[:, i, g, :])
                nc.vector.bn_aggr(out=muvar[:, g, i, :], in_=bs)

            def ln_veps(g, stats=True):
                """Per-head LN stats + veps -> inv_s via Ln/Exp (one ACT
                table set)."""
                if stats:
                    bs = small.tile(
                        [P, 8, nc.vector.BN_STATS_DIM], f32, tag="bs", name="bs"
                    )
                    for i in range(8):
                        nc.vector.bn_stats(
                            out=bs[:, i, :], in_=y_all[:, i, g, :]
                        )
                    for i in range(8):
                        nc.vector.bn_aggr(
                            out=muvar[:, g, i, :], in_=bs[:, i, :]
                        )
                dg = den_s[:, g, :]
                vg = muvar[:, g, :, 1:2]
                veps = small.tile([P, 8], f32, tag="veps", name="veps")
                nc.vector.tensor_mul(out=veps, in0=dg, in1=dg)
                nc.vector.tensor_scalar(
                    out=veps, in0=veps, scalar1=LN_EPS, scalar2=None,
                    op0=Alu.mult,
                )
                nc.vector.tensor_add(out=veps, in0=veps, in1=vg)
                ig = inv_s[:, g, :]
                nc.scalar.activation(out=ig, in_=veps, func=Act.Ln)
                nc.scalar.activation(
                    out=ig, in_=ig, func=Act.Exp, scale=-0.5, bias=lnb
                )

            def ln_apply(g, half):
                for i in range(4 * half, 4 * half + 4):
                    nc.vector.tensor_scalar(
                        out=y_all[:, i, g, :],
                        in0=y_all[:, i, g, :],
                        scalar1=muvar[:, g, i, 0:1],
                        scalar2=inv_s[:, g, i : i + 1],
                        op0=Alu.subtract,
                        op1=Alu.mult,
                    )

            # ---------- merged projections + attention ----------
            with (
                tc.tile_pool(name="xT", bufs=8) as xT_p,
                tc.tile_pool(name="w8", bufs=2) as w8_p,
                tc.tile_pool(name="qk", bufs=2) as qk_p,
                tc.tile_pool(name="outp", bufs=2) as out_p,
                tc.tile_pool(name="psS", bufs=1, space="PSUM") as psS,
                tc.tile_pool(name="psB2", bufs=2, space="PSUM") as psB2,
                tc.tile_pool(name="psY", bufs=2, space="PSUM") as psY,
            ):
                wv_ctx = tc.tile_pool(name="wv", bufs=8)
                wv_p = wv_ctx.__enter__()
                # critical path first: head-0 weights + xT on sync/scalar
                wt0 = w8_p.tile([P, NH, P], f16, tag="wq", name="wqh")
                nc.sync.dma_start(
                    out=wt0,
                    in_=wq_d.rearrange("(c p) d -> p c d", p=P)[:, :, 0:128],
                )
                wt1 = w8_p.tile([P, NH, P], f16, tag="wk", name="wkh")
                nc.scalar.dma_start(
                    out=wt1,
                    in_=wk_d.rearrange("(c p) d -> p c d", p=P)[:, :, 0:128],
                )
                wts = [wt0, wt1]
                xT = [xT_p.tile([P, T], f16, tag="xT", name="xT") for _ in range(8)]
                for c in range(8):
                    eng = (nc.sync, nc.scalar, nc.sync, nc.scalar,
                           nc.gpsimd, nc.gpsimd, nc.gpsimd, nc.gpsimd)[c]
                    eng.dma_start(out=xT[c], in_=xt_d[128 * c : 128 * (c + 1), :])

                # PE warm-up while the first DMAs land (pstate ramp)
                for _ in range(4):
                    scr = psB2.tile([P, 512], f32, tag="psB2", name="pps")
                    for w in range(4):
                        nc.tensor.matmul(
                            scr, lhsT=ident16, rhs=dum,
                            start=True, stop=True,
                        )

                wv_sb = [wv_p.tile([P, C], f16, tag="w", name="wsb") for _ in range(8)]
                for c in (2, 6, 3, 7, 0, 1, 4, 5):
                    eng = (nc.gpsimd, nc.gpsimd, nc.sync, nc.scalar)[c % 4]
                    eng.dma_start(
                        out=wv_sb[c], in_=wv_d[128 * c : 128 * (c + 1), :]
                    )
                nc.gpsimd.dma_start(out=lamneg, in_=lamneg_d[:, :])

                def emit_w8dma(h):
                    tiles = []
                    for w_d, tg, nm in ((wq_d, "wq", "wqh"), (wk_d, "wk", "wkh")):
                        wt = w8_p.tile([P, NH, P], f16, tag=tg, name=nm)
                        eng = nc.sync if tg == "wq" else nc.scalar
                        eng.dma_start(
                            out=wt,
                            in_=w_d.rearrange("(c p) d -> p c d", p=P)[
                                :, :, 128 * h : 128 * (h + 1)
                            ],
                        )
                        tiles.append(wt)
                    return tiles

                def emit_proj8(wt, dest):
                    """fp16 projection: (head_dim 128, T) for one head."""
                    for n in range(2):
                        ps = psB2.tile([P, 512], f32, tag="psB2", name="pps")
                        for c in range(8):
                            nc.tensor.matmul(
                                ps,
                                lhsT=wt[:, c, :],
                                rhs=xT[c][:, 512 * n : 512 * (n + 1)],
                                start=(c == 0),
                                stop=(c == 7),
                            )
                        nc.scalar.activation(
                            out=dest[:, 512 * n : 512 * (n + 1)], in_=ps,
                            func=Act.Copy,
                        )

                def score_unit(qT, kT, pcs, j):
                    """Scores for k-tile j, both streams in one [P,2,T]
                    PSUM tile (stream s in banks 2s..2s+1), ONE exp per j
                    over a strided 2-stream AP."""
                    sp = psS.tile([P, 2, T], f32, tag="psS", name="sp")
                    if j < 4:
                        for n in range(2):
                            for s in range(2):
                                nc.tensor.matmul(
                                    sp[:, s, max(128 * j, 512 * n) : 512 * (n + 1)],
                                    lhsT=kT[64 * s : 64 * (s + 1), 128 * j : 128 * (j + 1)],
                                    rhs=qT[64 * s : 64 * (s + 1), max(128 * j, 512 * n) : 512 * (n + 1)],
                                    start=True,
                                    stop=True,
                                )
                        nc.scalar.activation(
                            out=pcs[j][:, :, :],
                            in_=sp[:, :, 128 * j : 1024],
                            func=Act.Exp,
                            scale=SCORE_SCALE,
                        )
                    else:
                        qlo = 128 * (j - 4)
                        for s in range(2):
                            nc.tensor.matmul(
                                sp[:, s, qlo:512],
                                lhsT=kT[64 * s : 64 * (s + 1), 128 * j : 128 * (j + 1)],
                                rhs=qT[64 * s : 64 * (s + 1), 512 + qlo : 1024],
                                start=True,
                                stop=True,
                            )
                        nc.scalar.activation(
                            out=pcs[j][:, :, qlo:512],
                            in_=sp[:, :, qlo:512],
                            func=Act.Exp,
                            scale=SCORE_SCALE,
                        )
                    # causal mask on the diagonal 128-block of each stream
                    dlo = 0 if j < 4 else 128 * (j - 4)
                    for s in range(2):
                        nc.gpsimd.affine_select(
                            out=pcs[j][:, s, dlo : dlo + 128],
                            in_=pcs[j][:, s, dlo : dlo + 128],
                            compare_op=Alu.is_ge,
                            fill=0.0,
                            base=0,
                            pattern=[[1, 128]],
                            channel_multiplier=-1,
                        )

                def vproj_unit(t):
                    """V-projection for k-tile t, both 512-col halves per
                    xT chunk so each stationary is loaded once."""
                    pss = [
                        psB2.tile([P, 512], f32, tag="psB2", name="pps")
                        for _ in range(2)
                    ]
                    corder = (0, 1, 4, 5, 2, 3, 6, 7)  # wv DMA arrival order
                    for ci, c in enumerate(corder):
                        for n in range(2):
                            nc.tensor.matmul(
                                pss[n],
                                lhsT=xT[c][:, 128 * t : 128 * (t + 1)],
                                rhs=wv_sb[c][:, 512 * n : 512 * (n + 1)],
                                start=(ci == 0),
                                stop=(ci == 7),
                            )
                    nc.vector.tensor_copy(
                        out=v_aug[t][:, 0:4, 0:128],
                        in_=pss[0].rearrange("p (g d) -> p g d", g=4),
                    )
                    nc.scalar.activation(
                        out=v_aug[t][:, 4:8, 0:128],
                        in_=pss[1].rearrange("p (g d) -> p g d", g=4),
                        func=Act.Copy,
                    )
                    nc.vector.memset(v_aug[t][:, :, 128:129], 1.0)

                def trans_head(g):
                    """Transpose head g's LN'd column for all 8 q-tiles
                    into ylnT[:, g, :]. PSUM via psB2-tag f32 tiles bitcast
                    to f16 (no extra banks)."""
                    for half in range(2):
                        pt32 = psB2.tile([P, 256], f32, tag="psB2", name="pte")
                        pt = pt32.bitcast(f16)
                        for w in range(4):
                            i = 4 * half + w
                            nc.tensor.transpose(
                                out=pt[:, 128 * w : 128 * (w + 1)],
                                in_=y_all[:, i, g, :],
                                identity=ident16,
                            )
                        nc.scalar.activation(
                            out=ylnT[:, g, 512 * half : 512 * (half + 1)],
                            in_=pt, func=Act.Copy,
                        )

                wc_sb = []
                pcs_prev = None
                for h in range(NH):
                    if h == 2:
                        wv_ctx.__exit__(None, None, None)
                        wc_ctx = tc.tile_pool(name="wcp", bufs=8)
                        wc_p = wc_ctx.__enter__()
                        for d in range(8):
                            wct = wc_p.tile([P, C], f16, tag="wc", name="wcsb")
                            eng = nc.sync if d % 2 == 0 else nc.scalar
                            eng.dma_start(
                                out=wct, in_=wc_d[128 * d : 128 * (d + 1), :]
                            )
                            wc_sb.append(wct)
                    qT = qk_p.tile([P, T], f16, tag="q", name="qT")
                    kT = qk_p.tile([P, T], f16, tag="k", name="kT")
                    emit_proj8(wts[0], qT)
                    emit_proj8(wts[1], kT)
                    if h + 1 < NH:
                        next_wts = emit_w8dma(h + 1)
                    pcs = [
                        p_pool.tile(
                            [P, 2, (T - 128 * j) if j < 4 else 512],
                            f16, tag=f"p{j}", name="pch",
                        )
                        for j in range(8)
                    ]
                    yst = yst_p.tile([P, 8, 2, 130], f16, tag="yst", name="yst")
                    if h == 0:
                        backlog = [("v", t) for t in range(4)]
                    else:
                        backlog = [("pv", s, i) for i in range(8) for s in range(2)]
                        if h >= 2:
                            backlog.insert(2, ("lnv", h - 2))
                            backlog.insert(7, ("lna", h - 2, 0))
                            backlog.insert(12, ("lna", h - 2, 1))
                        if h >= 3:
                            backlog.insert(17, ("trh", h - 3))
                        backlog.append(("cs",))
                        backlog += [("cmb", i) for i in range(8)]
                        if h == 1:
                            backlog = [("v", 4), ("v", 5), ("v", 6), ("v", 7)] + backlog

                    gneg_h = [None]
                    ypair = {}

                    def do_unit(u):
                        if u[0] == "dum":
                            scr = psB2.tile([P, 512], f32, tag="psB2", name="pps")
                            for _ in range(4):
                                nc.tensor.matmul(
                                    scr, lhsT=ident16, rhs=dum,
                                    start=True, stop=True,
                                )
                        elif u[0] == "v":
                            vproj_unit(u[1])
                        elif u[0] == "pv":
                            pv_unit(
                                h - 1, u[1], u[2], pcs_prev, psY, yst_prev,
                                ypair,
                            )
                        elif u[0] == "cs":
                            gneg_h[0] = cmb_smalls(h - 1, yst_prev)
                        elif u[0] == "cmb":
                            cmb_unit(h - 1, u[1], yst_prev, gneg_h[0])
                        elif u[0] == "lnv":
                            ln_veps(u[1])
                        elif u[0] == "trh":
                            trans_head(u[1])
                        else:
                            ln_apply(u[1], u[2])

                    done = 0
                    for j in range(8):
                        score_unit(qT, kT, pcs, j)
                        while done < len(backlog) and (j + 1) * len(
                            backlog
                        ) >= (done + 1) * 8:
                            do_unit(backlog[done])
                            done += 1
                    while done < len(backlog):
                        do_unit(backlog[done])
                        done += 1
                    pcs_prev = pcs
                    yst_prev = yst
                    if h + 1 < NH:
                        wts = next_wts

            # ---------- tail: PV(7) + LN(6,7) + transpose + c_proj ----------
                def emit_cproj(i):
                    osb = out_p.tile([P, C], f16, tag="osb")
                    if i % 2 == 0:
                        big = psS.tile([P, 2, T], f32, tag="psS", name="psf")
                        pss = [big[:, 0, 0:512], big[:, 1, 0:512]]
                    else:
                        pss = [
                            psB2.tile([P, 512], f32, tag="psB2", name="psf")
                            for _ in range(2)
                        ]
                    for d in range(8):
                        for n in range(2):
                            nc.tensor.matmul(
                                pss[n],
                                lhsT=ylnT[:, d, 128 * i : 128 * (i + 1)],
                                rhs=wc_sb[d][:, 512 * n : 512 * (n + 1)],
                                start=(d == 0),
                                stop=(d == 7),
                            )
                    for n in range(2):
                        nc.vector.tensor_copy(
                            out=osb[:, 512 * n : 512 * (n + 1)], in_=pss[n]
                        )
                    eng = nc.sync if i % 2 == 0 else nc.scalar
                    eng.dma_start(out=out_d[128 * i : 128 * (i + 1), :], in_=osb)

                # PV(7) with LN(6) and early transposes (heads 0-3) mixed in
                yst7 = yst_p.tile([P, 8, 2, 130], f16, tag="yst", name="yst")
                trans_head(5)
                ypair7 = {}
                tail_units = []
                for i in range(8):
                    tail_units.append(("pv", 0, i))
                    tail_units.append(("pv", 1, i))
                tail_units.insert(4, ("lnv", 6))
                tail_units.insert(9, ("lna", 6, 0))
                tail_units.insert(14, ("lna", 6, 1))
                for u in tail_units:
                    if u[0] == "pv":
                        pv_unit(7, u[1], u[2], pcs_prev, psY, yst7, ypair7)
                        if u[1] == 1:
                            cmb_now(7, u[2], yst7)
                            ln_fin(7, u[2])
                    elif u[0] == "lnv":
                        ln_veps(u[1])
                    else:
                        ln_apply(u[1], u[2])
                    if u[0] == "lna" and u[1] == 6 and u[2] == 1:
                        trans_head(6)
                for half in range(2):
                    pt32a = psB2.tile([P, 256], f32, tag="psB2", name="pte")
                    pta = pt32a.bitcast(f16)
                    for w in range(4):
                        nc.tensor.transpose(
                            out=pta[:, 128 * w : 128 * (w + 1)],
                            in_=y_all[:, 4 * half + w, 7, :],
                            identity=ident16,
                        )
                    nc.vector.tensor_copy(
                        out=ylnT[:, 7, 512 * half : 512 * (half + 1)], in_=pta
                    )
                    for i in range(4 * half, 4 * half + 4):
                        emit_cproj(i)
                wc_ctx.__exit__(None, None, None)

            small_ctx.__exit__(None, None, None)
            p_ctx.__exit__(None, None, None)
            yst_ctx.__exit__(None, None, None)

    bass._bass_rust.generate_event_semaphores(nc)
    return nc


# revision 45
# speedup vs baseline: 1.0161x; 1.0161x over previous
"""MultiHeadDiffAttention Trainium2 kernel (8 NeuronCores), v3.

Sharding: batch (4) x head-group (2 groups of 8 heads) = 8 cores.
Each core computes a partial (T, C) c_proj output for its batch element
restricted to its 8 heads; the host sums the two head-group partials
(fp16 partials, fp32 host sum).

~230us HW (v1 baseline: ~264us). All matmuls fp16 (fp8 DoubleRow was
tried for Q/K projections: 2x PE rate but the differential-attention
cancellation amplifies quantization noise ~4x -> 3.7e-2 rel err, FAIL).

Key structure:
  - Scores per (stream, k-tile j) into one [P,2,1024] f32 PSUM tile
    (stream s in banks 2s..2s+1), ONE merged exp ACT per j (strided
    2-stream AP for j<4) -> 8-15 exps/head instead of 24; P chunks
    stored tightly per-j ([P,2,1024-128j] fp16, q-offset layout for
    j>=4).
  - Dual-stream trick: PV with a ones column yields Y_s and softmax
    denominator den_s per stream; z = Y1 - (lam*den1/den2)*Y2 equals
    den1*(a1 - lam*a2)@V, and LN scale-invariance (eps scaled by
    den1^2, folded into the Ln/Exp chain with bias ln(1-lambda))
    reproduces the reference exactly.
  - PV results staged PSUM->SBUF fp16 immediately (frees banks, the
    combine is batched per head on DVE with a single recip/gneg batch).
  - LN stats (bn_stats/aggr) + veps + apply batched per head, run in
    the main loop two heads behind; per-head PE transposes (fp16
    identity) three heads behind -> tiny tail.
  - Tail: PV(7) with per-tile combine+LN finish, head-7 transposes per
    half interleaved with c_proj (c_proj PSUM alternates psS/psB2
    banks); fp16 output DMA per q-tile on alternating queues.
  - In-order PE queue discipline: warm-up dummies sized to the pstate
    ramp (~3.5us) only; input DMAs ordered wq/wk head0 -> xT -> wv
    split across sync/scalar/gpsimd queues.
"""

import contextlib
import ctypes
import math
import sys
import types

import numpy as np

sys.path.insert(0, "/opt/trn_rl_repo")


def _install_ntff_hook():
    """Provide antenv.axon_hooks if the image lacks it (for trace=True)."""
    try:
        from antenv.axon_hooks import get_axon_ntff_profile_hook  # noqa: F401

        return
    except ImportError:
        pass

    so_path = "/opt/axon/libaxon_pjrt.so"

    def _make_hook():
        try:
            lib = ctypes.CDLL(so_path)
        except OSError:
            return None
        if not hasattr(lib, "axon_start_nrt_profile"):
            return None
        lib.axon_start_nrt_profile.argtypes = [
            ctypes.POINTER(ctypes.c_int64),
            ctypes.c_size_t,
        ]
        lib.axon_start_nrt_profile.restype = ctypes.c_int64
        lib.axon_stop_nrt_profile.argtypes = [ctypes.c_char_p]
        lib.axon_stop_nrt_profile.restype = ctypes.c_int64

        @contextlib.contextmanager
        def _hook(output_dir, device_ids):
            import jax

            jax.devices()
            if device_ids:
                ids = (ctypes.c_int64 * len(device_ids))(*device_ids)
                rc = lib.axon_start_nrt_profile(ids, len(device_ids))
            else:
                rc = lib.axon_start_nrt_profile(None, 0)
            if rc != 0:
                raise RuntimeError(f"axon_start_nrt_profile rc={rc}")
            try:
                yield
            finally:
                n = lib.axon_stop_nrt_profile(str(output_dir).encode())
                if n < 0:
                    raise RuntimeError(f"axon_stop_nrt_profile rc={n}")

        return _hook

    mod = types.ModuleType("antenv.axon_hooks")
    _the_hook = _make_hook()
    mod.get_axon_ntff_profile_hook = lambda: _the_hook
    sys.modules["antenv.axon_hooks"] = mod


_install_ntff_hook()

import ml_dtypes  # noqa: E402

import concourse.bass as bass  # noqa: E402
import concourse.mybir as mybir  # noqa: E402
import concourse.tile as tile  # noqa: E402
from concourse.masks import make_identity  # noqa: E402

P = 128
T = 1024
C = 1024
NH = 8  # heads per core
HS = 64
LAMBDA_INIT = 0.8 - 0.6 * math.exp(-0.3 * (2 - 1))
LN_EPS = 1e-5
N_CORES = 8
SCORE_SCALE = 0.125

f32 = mybir.dt.float32
f16 = mybir.dt.float16
f8 = mybir.dt.float8e4
Alu = mybir.AluOpType
Act = mybir.ActivationFunctionType
DR = mybir.MatmulPerfMode.DoubleRow


def build_program():
    nc = bass.Bass()
    xt_d = nc.dram_tensor("xt", [C, T], f16, kind="ExternalInput")
    wq_d = nc.dram_tensor("wq", [C, C], f16, kind="ExternalInput")
    wk_d = nc.dram_tensor("wk", [C, C], f16, kind="ExternalInput")
    wv_d = nc.dram_tensor("wv", [C, C], f16, kind="ExternalInput")
    wc_d = nc.dram_tensor("wc", [C, C], f16, kind="ExternalInput")
    lamneg_d = nc.dram_tensor("lamneg", [P, NH], f32, kind="ExternalInput")
    out_d = nc.dram_tensor("out", [T, C], f16, kind="ExternalOutput")

    ln_bias = float(math.log(1.0 - LAMBDA_INIT))

    with tile.TileContext(nc) as tc:
        with (
            tc.tile_pool(name="const", bufs=1) as const,
            tc.tile_pool(name="ydata", bufs=1) as y_pool,
            tc.tile_pool(name="vdata", bufs=8) as v_p,
            tc.tile_pool(name="ylnTp", bufs=1) as ylnT_p,
        ):
            ident16 = const.tile([P, P], f16, tag="ident")
            make_identity(nc, ident16)
            lamneg = const.tile([P, NH], f32, tag="lamneg")
            den_s = const.tile([P, NH, 8], f32, tag="den")
            muvar = const.tile([P, NH, 8, 2], f32, tag="muvar")
            inv_s = const.tile([P, NH, 8], f32, tag="inv")
            lnb = const.tile([P, 1], f32, tag="lnb")
            nc.vector.memset(lnb, ln_bias)
            dum = const.tile([P, 512], f16, tag="dum")
            nc.vector.memset(dum, 0.0)

            y_all = y_pool.tile([P, 8, NH, P], f16, tag="y", name="yall")
            v_aug = [
                v_p.tile([P, NH, 130], f16, tag="v", name="vaug") for _ in range(8)
            ]
            ylnT = ylnT_p.tile([P, NH, T], f16, tag="ylnT")

            yst_ctx = tc.tile_pool(name="ystp", bufs=1)
            yst_p = yst_ctx.__enter__()
            p_ctx = tc.tile_pool(name="pprob", bufs=2)
            p_pool = p_ctx.__enter__()
            small_ctx = tc.tile_pool(name="smallc", bufs=4)
            small = small_ctx.__enter__()

            def pv_unit(h, s, i, pcs, ypool, yst, ypair):
                """PV for one (stream, q-tile): accumulate over k-tiles with
                the ones column giving the softmax denominator, then stage
                the PSUM result to SBUF fp16. Both streams of a q-tile
                share one [P,2,129] PSUM tile so 2 pool bufs give pipeline
                depth 4."""
                yp = ypool.tile([P, 129], f32, tag="psY", name="yp")
                for j in range(i + 1):
                    pj = pcs[j]
                    lo = 128 * (i - j) if j < 4 else 128 * i - 512
                    nc.tensor.matmul(
                        yp,
                        lhsT=pj[:, s, lo : lo + 128],
                        rhs=v_aug[j][:, h, 0:129],
                        start=(j == 0),
                        stop=(j == i),
                    )
                nc.vector.tensor_copy(out=yst[:, i, s, 0:129], in_=yp)

            def cmb_smalls(g, yst):
                """Batched per-head: gneg_i = -lam*den1_i/den2_i, and den1
                saved for the veps chain."""
                nc.vector.tensor_copy(
                    out=den_s[:, g, :], in_=yst[:, :, 0, 128:129]
                )
                r2 = small.tile([P, 8], f32, tag="r2", name="r2")
                nc.vector.reciprocal(out=r2, in_=yst[:, :, 1, 128:129])
                nc.vector.tensor_mul(out=r2, in0=r2, in1=den_s[:, g, :])
                gneg = small.tile([P, 8], f32, tag="gneg", name="gneg")
                nc.vector.tensor_scalar(
                    out=gneg, in0=r2, scalar1=lamneg[:, g : g + 1],
                    scalar2=None, op0=Alu.mult,
                )
                return gneg

            def cmb_unit(g, i, yst, gneg):
                """z_i = y1_i + gneg_i * y2_i -> y_all."""
                tmp = small.tile([P, P], f16, tag="tmp", name="tmp")
                nc.vector.tensor_scalar(
                    out=tmp, in0=yst[:, i, 1, 0:128],
                    scalar1=gneg[:, i : i + 1], scalar2=None, op0=Alu.mult,
                )
                nc.vector.tensor_add(
                    out=y_all[:, i, g, :], in0=yst[:, i, 0, 0:128], in1=tmp
                )

            def ln_fin(g, i):
                """Per-tile veps -> inv -> LN apply (tail fast path)."""
                veps = small.tile([P, 1], f32, tag="veps1", name="veps1")
                nc.vector.tensor_mul(
                    out=veps, in0=den_s[:, g, i : i + 1],
                    in1=den_s[:, g, i : i + 1],
                )
                nc.vector.tensor_scalar(
                    out=veps, in0=veps, scalar1=LN_EPS, scalar2=None,
                    op0=Alu.mult,
                )
                nc.vector.tensor_add(
                    out=veps, in0=veps, in1=muvar[:, g, i, 1:2]
                )
                ig = inv_s[:, g, i : i + 1]
                nc.scalar.activation(out=ig, in_=veps, func=Act.Ln)
                nc.scalar.activation(
                    out=ig, in_=ig, func=Act.Exp, scale=-0.5, bias=lnb
                )
                nc.vector.tensor_scalar(
                    out=y_all[:, i, g, :],
                    in0=y_all[:, i, g, :],
                    scalar1=muvar[:, g, i, 0:1],
                    scalar2=ig,
                    op0=Alu.subtract,
                    op1=Alu.mult,
                )

            def cmb_now(g, i, yst):
                """Per-tile combine + LN stats (tail path: no batching so
                the chain overlaps the remaining PV units)."""
                nc.vector.tensor_copy(
                    out=den_s[:, g, i : i + 1], in_=yst[:, i, 0, 128:129]
                )
                r2 = small.tile([P, 1], f32, tag="r2", name="r2")
                nc.vector.reciprocal(out=r2, in_=yst[:, i, 1, 128:129])
                gneg = small.tile([P, 1], f32, tag="gneg", name="gneg")
                nc.vector.tensor_scalar(
                    out=gneg, in0=r2, scalar1=den_s[:, g, i : i + 1],
                    scalar2=lamneg[:, g : g + 1], op0=Alu.mult, op1=Alu.mult,
                )
                tmp = small.tile([P, P], f16, tag="tmp", name="tmp")
                nc.vector.tensor_scalar(
                    out=tmp, in0=yst[:, i, 1, 0:128],
                    scalar1=gneg, scalar2=None, op0=Alu.mult,
                )
                nc.vector.tensor_add(
                    out=y_all[:, i, g, :], in0=yst[:, i, 0, 0:128], in1=tmp
                )
                bs = small.tile(
                    [P, nc.vector.BN_STATS_DIM], f32, tag="bs1", name="bs1"
                )
                nc.vector.bn_stats(out=bs, in_=y_all[:, i, g, :])
                nc.vector.bn_aggr(out=muvar[:, g, i, :], in_=bs)

            def ln_veps(g, stats=True):
                """Per-head LN stats + veps -> inv_s via Ln/Exp (one ACT
                table set)."""
                if stats:
                    bs = small.tile(
                        [P, 8, nc.vector.BN_STATS_DIM], f32, tag="bs", name="bs"
                    )
                    for i in range(8):
                        nc.vector.bn_stats(
                            out=bs[:, i, :], in_=y_all[:, i, g, :]
                        )
                    for i in range(8):
                        nc.vector.bn_aggr(
                            out=muvar[:, g, i, :], in_=bs[:, i, :]
                        )
                dg = den_s[:, g, :]
                vg = muvar[:, g, :, 1:2]
                veps = small.tile([P, 8], f32, tag="veps", name="veps")
                nc.vector.tensor_mul(out=veps, in0=dg, in1=dg)
                nc.vector.tensor_scalar(
                    out=veps, in0=veps, scalar1=LN_EPS, scalar2=None,
                    op0=Alu.mult,
                )
                nc.vector.tensor_add(out=veps, in0=veps, in1=vg)
                ig = inv_s[:, g, :]
                nc.scalar.activation(out=ig, in_=veps, func=Act.Ln)
                nc.scalar.activation(
                    out=ig, in_=ig, func=Act.Exp, scale=-0.5, bias=lnb
                )

            def ln_apply(g, half):
                for i in range(4 * half, 4 * half + 4):
                    nc.vector.tensor_scalar(
                        out=y_all[:, i, g, :],
                        in0=y_all[:, i, g, :],
                        scalar1=muvar[:, g, i, 0:1],
                        scalar2=inv_s[:, g, i : i + 1],
                        op0=Alu.subtract,
                        op1=Alu.mult,
                    )

            # ---------- merged projections + attention ----------
            with (
                tc.tile_pool(name="xT", bufs=8) as xT_p,
                tc.tile_pool(name="w8", bufs=2) as w8_p,
                tc.tile_pool(name="qk", bufs=2) as qk_p,
                tc.tile_pool(name="outp", bufs=2) as out_p,
                tc.tile_pool(name="psS", bufs=1, space="PSUM") as psS,
                tc.tile_pool(name="psB2", bufs=2, space="PSUM") as psB2,
                tc.tile_pool(name="psY", bufs=2, space="PSUM") as psY,
            ):
                wv_ctx = tc.tile_pool(name="wv", bufs=8)
                wv_p = wv_ctx.__enter__()
                # critical path first: head-0 weights + xT on sync/scalar
                wt0 = w8_p.tile([P, NH, P], f16, tag="wq", name="wqh")
                nc.sync.dma_start(
                    out=wt0,
                    in_=wq_d.rearrange("(c p) d -> p c d", p=P)[:, :, 0:128],
                )
                wt1 = w8_p.tile([P, NH, P], f16, tag="wk", name="wkh")
                nc.scalar.dma_start(
                    out=wt1,
                    in_=wk_d.rearrange("(c p) d -> p c d", p=P)[:, :, 0:128],
                )
                wts = [wt0, wt1]
                xT = [xT_p.tile([P, T], f16, tag="xT", name="xT") for _ in range(8)]
                for c in range(8):
                    eng = nc.sync if c % 2 == 0 else nc.scalar
                    eng.dma_start(out=xT[c], in_=xt_d[128 * c : 128 * (c + 1), :])

                # PE warm-up while the first DMAs land (pstate ramp)
                for _ in range(4):
                    scr = psB2.tile([P, 512], f32, tag="psB2", name="pps")
                    for w in range(4):
                        nc.tensor.matmul(
                            scr, lhsT=ident16, rhs=dum,
                            start=True, stop=True,
                        )

                wv_sb = [wv_p.tile([P, C], f16, tag="w", name="wsb") for _ in range(8)]
                for c in range(8):
                    eng = (nc.gpsimd, nc.gpsimd, nc.sync, nc.scalar)[c % 4]
                    eng.dma_start(
                        out=wv_sb[c], in_=wv_d[128 * c : 128 * (c + 1), :]
                    )
                nc.gpsimd.dma_start(out=lamneg, in_=lamneg_d[:, :])

                def emit_w8dma(h):
                    tiles = []
                    for w_d, tg, nm in ((wq_d, "wq", "wqh"), (wk_d, "wk", "wkh")):
                        wt = w8_p.tile([P, NH, P], f16, tag=tg, name=nm)
                        eng = nc.sync if tg == "wq" else nc.scalar
                        eng.dma_start(
                            out=wt,
                            in_=w_d.rearrange("(c p) d -> p c d", p=P)[
                                :, :, 128 * h : 128 * (h + 1)
                            ],
                        )
                        tiles.append(wt)
                    return tiles

                def emit_proj8(wt, dest):
                    """fp16 projection: (head_dim 128, T) for one head."""
                    for n in range(2):
                        ps = psB2.tile([P, 512], f32, tag="psB2", name="pps")
                        for c in range(8):
                            nc.tensor.matmul(
                                ps,
                                lhsT=wt[:, c, :],
                                rhs=xT[c][:, 512 * n : 512 * (n + 1)],
                                start=(c == 0),
                                stop=(c == 7),
                            )
                        nc.scalar.activation(
                            out=dest[:, 512 * n : 512 * (n + 1)], in_=ps,
                            func=Act.Copy,
                        )

                def score_unit(qT, kT, pcs, j):
                    """Scores for k-tile j, both streams in one [P,2,T]
                    PSUM tile (stream s in banks 2s..2s+1), ONE exp per j
                    over a strided 2-stream AP."""
                    sp = psS.tile([P, 2, T], f32, tag="psS", name="sp")
                    if j < 4:
                        for n in range(2):
                            for s in range(2):
                                nc.tensor.matmul(
                                    sp[:, s, max(128 * j, 512 * n) : 512 * (n + 1)],
                                    lhsT=kT[64 * s : 64 * (s + 1), 128 * j : 128 * (j + 1)],
                                    rhs=qT[64 * s : 64 * (s + 1), max(128 * j, 512 * n) : 512 * (n + 1)],
                                    start=True,
                                    stop=True,
                                )
                        nc.scalar.activation(
                            out=pcs[j][:, :, :],
                            in_=sp[:, :, 128 * j : 1024],
                            func=Act.Exp,
                            scale=SCORE_SCALE,
                        )
                    else:
                        qlo = 128 * (j - 4)
                        for s in range(2):
                            nc.tensor.matmul(
                                sp[:, s, qlo:512],
                                lhsT=kT[64 * s : 64 * (s + 1), 128 * j : 128 * (j + 1)],
                                rhs=qT[64 * s : 64 * (s + 1), 512 + qlo : 1024],
                                start=True,
                                stop=True,
                            )
                        nc.scalar.activation(
                            out=pcs[j][:, :, qlo:512],
                            in_=sp[:, :, qlo:512],
                            func=Act.Exp,
                            scale=SCORE_SCALE,
                        )
                    # causal mask on the diagonal 128-block of each stream
                    dlo = 0 if j < 4 else 128 * (j - 4)
                    for s in range(2):
                        nc.gpsimd.affine_select(
                            out=pcs[j][:, s, dlo : dlo + 128],
                            in_=pcs[j][:, s, dlo : dlo + 128],
                            compare_op=Alu.is_ge,
                            fill=0.0,
                            base=0,
                            pattern=[[1, 128]],
                            channel_multiplier=-1,
                        )

                def vproj_unit(t):
                    """V-projection for k-tile t, both 512-col halves per
                    xT chunk so each stationary is loaded once."""
                    pss = [
                        psB2.tile([P, 512], f32, tag="psB2", name="pps")
                        for _ in range(2)
                    ]
                    corder = (0, 1, 4, 5, 2, 3, 6, 7)  # wv DMA arrival order
                    for ci, c in enumerate(corder):
                        for n in range(2):
                            nc.tensor.matmul(
                                pss[n],
                                lhsT=xT[c][:, 128 * t : 128 * (t + 1)],
                                rhs=wv_sb[c][:, 512 * n : 512 * (n + 1)],
                                start=(ci == 0),
                                stop=(ci == 7),
                            )
                    nc.vector.tensor_copy(
                        out=v_aug[t][:, 0:4, 0:128],
                        in_=pss[0].rearrange("p (g d) -> p g d", g=4),
                    )
                    nc.scalar.activation(
                        out=v_aug[t][:, 4:8, 0:128],
                        in_=pss[1].rearrange("p (g d) -> p g d", g=4),
                        func=Act.Copy,
                    )
                    nc.vector.memset(v_aug[t][:, :, 128:129], 1.0)

                def trans_head(g):
                    """Transpose head g's LN'd column for all 8 q-tiles
                    into ylnT[:, g, :]. PSUM via psB2-tag f32 tiles bitcast
                    to f16 (no extra banks)."""
                    for half in range(2):
                        pt32 = psB2.tile([P, 256], f32, tag="psB2", name="pte")
                        pt = pt32.bitcast(f16)
                        for w in range(4):
                            i = 4 * half + w
                            nc.tensor.transpose(
                                out=pt[:, 128 * w : 128 * (w + 1)],
                                in_=y_all[:, i, g, :],
                                identity=ident16,
                            )
                        nc.scalar.activation(
                            out=ylnT[:, g, 512 * half : 512 * (half + 1)],
                            in_=pt, func=Act.Copy,
                        )

                wc_sb = []
                pcs_prev = None
                for h in range(NH):
                    if h == 2:
                        wv_ctx.__exit__(None, None, None)
                        wc_ctx = tc.tile_pool(name="wcp", bufs=8)
                        wc_p = wc_ctx.__enter__()
                        for d in range(8):
                            wct = wc_p.tile([P, C], f16, tag="wc", name="wcsb")
                            eng = nc.sync if d % 2 == 0 else nc.scalar
                            eng.dma_start(
                                out=wct, in_=wc_d[128 * d : 128 * (d + 1), :]
                            )
                            wc_sb.append(wct)
                    qT = qk_p.tile([P, T], f16, tag="q", name="qT")
                    kT = qk_p.tile([P, T], f16, tag="k", name="kT")
                    emit_proj8(wts[0], qT)
                    emit_proj8(wts[1], kT)
                    if h + 1 < NH:
                        next_wts = emit_w8dma(h + 1)
                    pcs = [
                        p_pool.tile(
                            [P, 2, (T - 128 * j) if j < 4 else 512],
                            f16, tag=f"p{j}", name="pch",
                        )
                        for j in range(8)
                    ]
                    yst = yst_p.tile([P, 8, 2, 130], f16, tag="yst", name="yst")
                    if h == 0:
                        backlog = [("v", t) for t in range(4)]
                    else:
                        backlog = [("pv", s, i) for i in range(8) for s in range(2)]
                        if h >= 2:
                            backlog.insert(2, ("lnv", h - 2))
                            backlog.insert(7, ("lna", h - 2, 0))
                            backlog.insert(12, ("lna", h - 2, 1))
                        if h >= 3:
                            backlog.insert(17, ("trh", h - 3))
                        backlog.append(("cs",))
                        backlog += [("cmb", i) for i in range(8)]
                        if h == 1:
                            backlog = [("v", 4), ("v", 5), ("v", 6), ("v", 7)] + backlog

                    gneg_h = [None]
                    ypair = {}

                    def do_unit(u):
                        if u[0] == "dum":
                            scr = psB2.tile([P, 512], f32, tag="psB2", name="pps")
                            for _ in range(4):
                                nc.tensor.matmul(
                                    scr, lhsT=ident16, rhs=dum,
                                    start=True, stop=True,
                                )
                        elif u[0] == "v":
                            vproj_unit(u[1])
                        elif u[0] == "pv":
                            pv_unit(
                                h - 1, u[1], u[2], pcs_prev, psY, yst_prev,
                                ypair,
                            )
                        elif u[0] == "cs":
                            gneg_h[0] = cmb_smalls(h - 1, yst_prev)
                        elif u[0] == "cmb":
                            cmb_unit(h - 1, u[1], yst_prev, gneg_h[0])
                        elif u[0] == "lnv":
                            ln_veps(u[1])
                        elif u[0] == "trh":
                            trans_head(u[1])
                        else:
                            ln_apply(u[1], u[2])

                    done = 0
                    for j in range(8):
                        score_unit(qT, kT, pcs, j)
                        while done < len(backlog) and (j + 1) * len(
                            backlog
                        ) >= (done + 1) * 8:
                            do_unit(backlog[done])
                            done += 1
                    while done < len(backlog):
                        do_unit(backlog[done])
                        done += 1
                    pcs_prev = pcs
                    yst_prev = yst
                    if h + 1 < NH:
                        wts = next_wts

            # ---------- tail: PV(7) + LN(6,7) + transpose + c_proj ----------
                def emit_cproj(i):
                    osb = out_p.tile([P, C], f16, tag="osb")
                    if i % 2 == 0:
                        big = psS.tile([P, 2, T], f32, tag="psS", name="psf")
                        pss = [big[:, 0, 0:512], big[:, 1, 0:512]]
                    else:
                        pss = [
                            psB2.tile([P, 512], f32, tag="psB2", name="psf")
                            for _ in range(2)
                        ]
                    for d in range(8):
                        for n in range(2):
                            nc.tensor.matmul(
                                pss[n],
                                lhsT=ylnT[:, d, 128 * i : 128 * (i + 1)],
                                rhs=wc_sb[d][:, 512 * n : 512 * (n + 1)],
                                start=(d == 0),
                                stop=(d == 7),
                            )
                    for n in range(2):
                        nc.vector.tensor_copy(
                            out=osb[:, 512 * n : 512 * (n + 1)], in_=pss[n]
                        )
                    eng = nc.sync if i % 2 == 0 else nc.scalar
                    eng.dma_start(out=out_d[128 * i : 128 * (i + 1), :], in_=osb)

                # PV(7) with LN(6) and early transposes (heads 0-3) mixed in
                yst7 = yst_p.tile([P, 8, 2, 130], f16, tag="yst", name="yst")
                trans_head(5)
                ypair7 = {}
                tail_units = []
                for i in range(8):
                    tail_units.append(("pv", 0, i))
                    tail_units.append(("pv", 1, i))
                tail_units.insert(4, ("lnv", 6))
                tail_units.insert(9, ("lna", 6, 0))
                tail_units.insert(14, ("lna", 6, 1))
                for u in tail_units:
                    if u[0] == "pv":
                        pv_unit(7, u[1], u[2], pcs_prev, psY, yst7, ypair7)
                        if u[1] == 1:
                            cmb_now(7, u[2], yst7)
                            ln_fin(7, u[2])
                    elif u[0] == "lnv":
                        ln_veps(u[1])
                    else:
                        ln_apply(u[1], u[2])
                    if u[0] == "lna" and u[1] == 6 and u[2] == 1:
                        trans_head(6)
                for half in range(2):
                    pt32a = psB2.tile([P, 256], f32, tag="psB2", name="pte")
                    pta = pt32a.bitcast(f16)
                    for w in range(4):
                        nc.tensor.transpose(
                            out=pta[:, 128 * w : 128 * (w + 1)],
                            in_=y_all[:, 4 * half + w, 7, :],
                            identity=ident16,
                        )
                    nc.vector.tensor_copy(
                        out=ylnT[:, 7, 512 * half : 512 * (half + 1)], in_=pta
                    )
                    for i in range(4 * half, 4 * half + 4):
                        emit_cproj(i)
                wc_ctx.__exit__(None, None, None)

            small_ctx.__exit__(None, None, None)
            p_ctx.__exit__(None, None, None)
            yst_ctx.__exit__(None, None, None)

    bass._bass_rust.generate_event_semaphores(nc)
    return nc
